# revision 1
# baseline (speedup 1.0000x reference)
"""Bidirectional LSTM (B=64, T=512, D=512, H=1024) on 8 TRN2 NeuronCores.

Strategy:
  - Tensor-parallel over the gate/channel dim: core r owns channels
    [128r, 128r+128) of each gate (i,f,o,g), for BOTH directions.
  - Full batch (64) on every core. Forward and backward direction are packed
    into the 128 PSUM partitions (rows 0:64 = fwd batch, 64:128 = bwd batch)
    via PE column-tiling, so the 128x128 array is fully used.
  - Per step: z = x_t @ Wx + h_{t-1} @ Wh computed with h^T/x^T as the
    128x128 stationary operand (batch in stationary columns) and the weight
    shard [128, 512] as the moving operand (bf16, 1 col/cycle).
  - Gates: sigmoid/tanh on ScalarE (same ACT table set), c/h updates on DVE.
  - h is PE-transposed to h^T [ch, batch], cast to bf16, and exchanged with
    the other 7 cores via an AllGather each step; the gathered chunks are the
    next step's matmul stationaries.
  - Backward direction runs on reversed time inside the same step loop.

bf16 matmul inputs with fp32 PSUM accumulation give rel_err ~2.4e-3 vs the
fp32 reference (measured in numpy simulation), well under the 2e-2 gate.

Exchange modes (LSTM_EXCHANGE env, default "collective"):
  - "collective": ncfw AllGather per step (~10.8 ms device time). Stable
    across arbitrarily many back-to-back executions — this is the default.
  - "remote": SBUF->SBUF remote DMA exchange (~2x faster per step), but the
    axon/SWDGE stack shows stochastic "mesh desynced" failures (~1 in 5-15
    executions), after which the WHOLE in-process mesh is dead (even the
    collective fallback fails). Opt-in only.
  - "none": timing-only skeleton (wrong results), for perf experiments.
"""

import os
import numpy as np
import ml_dtypes

import concourse.bacc as bacc
import concourse.mybir as mybir
from concourse.tile import TileContext
from concourse.tile_rust import add_dep_helper
from concourse.masks import make_identity
from concourse.bass_utils import run_bass_kernel_spmd

BF16 = ml_dtypes.bfloat16

B, T, D, H = 64, 512, 512, 1024
NC = 8
CH = H // NC            # channels per core per gate (128)
SH = 4 * CH             # shard width per direction (512)
KC_H = H // 128         # h contraction chunks (8)
KC_X = D // 128         # x contraction chunks (4)
STORE_EVERY = 8         # steps of h accumulated in SBUF before one out-DMA

bf = mybir.dt.bfloat16
f32 = mybir.dt.float32
AF = mybir.ActivationFunctionType
ALU = mybir.AluOpType


def build_nc(Tsteps=T, with_bias=False, exchange="collective"):
    nc = bacc.Bacc(
        None, target_bir_lowering=False, debug=False, num_devices=NC,
        # the sim race detector can't model pipelined cross-core sem
        # increments (level-triggered >= waits make them benign)
        detect_race_conditions=(exchange != "remote"),
    )

    xT = nc.declare_dram_parameter("xT", [128, Tsteps, KC_X, B], bf, isOutput=False)
    wh = nc.declare_dram_parameter("wh", [128, 2, KC_H, SH], bf, isOutput=False)
    wx = nc.declare_dram_parameter("wx", [128, 2, KC_X, SH], bf, isOutput=False)
    if with_bias:
        bias = nc.declare_dram_parameter("bias", [1, 2, SH], bf, isOutput=False)
    out = nc.declare_dram_parameter("out", [128, Tsteps, CH], f32, isOutput=True)

    with TileContext(nc) as tc:
        with (
            tc.tile_pool(name="const", bufs=1) as const_pool,
            tc.tile_pool(name="state", bufs=1) as state_pool,
            tc.tile_pool(name="xin", bufs=4) as x_pool,
            tc.tile_pool(name="work", bufs=2) as work_pool,
            tc.tile_pool(name="hT", bufs=2) as hT_pool,
            tc.tile_pool(name="zps", bufs=2, space="PSUM") as psum_pool,
            tc.tile_pool(name="tps", bufs=2, space="PSUM") as tpsum_pool,
            tc.tile_pool(name="outb", bufs=2) as out_pool,
            tc.tile_pool(name="dram", bufs=2, space="DRAM") as dram_pool,
        ):
            wh_sb = const_pool.tile([128, 2, KC_H, SH], bf)
            nc.sync.dma_start(out=wh_sb[:], in_=wh[:, :, :, :])
            wx_sb = const_pool.tile([128, 2, KC_X, SH], bf)
            nc.sync.dma_start(out=wx_sb[:], in_=wx[:, :, :, :])
            ident = const_pool.tile([128, 128], f32)
            make_identity(nc, ident[:])
            if with_bias:
                ones_sb = const_pool.tile([1, 128], bf)
                nc.gpsimd.memset(ones_sb[:], 1.0)
                bias_sb = const_pool.tile([1, 2, SH], bf)
                nc.sync.dma_start(out=bias_sb[:], in_=bias[:, :, :])

            c_st = state_pool.tile([128, CH], f32)
            nc.gpsimd.memset(c_st[:], 0.0)

            if exchange == "remote":
                arrive_sem = nc.alloc_semaphore("hT_arrive")
                send_sem = nc.alloc_semaphore("hT_send")
                # remote_sem credit per sender per step: 16 // n_dest_slots
                ARR_INC = (NC - 1) * (16 // NC)
                # manually allocated sems start with garbage; clear them, then
                # barrier (a core leaving the AllReduce implies every core has
                # entered it, i.e. finished its clears) before any send
                c1 = nc.gpsimd.sem_clear(arrive_sem)
                c2 = nc.gpsimd.sem_clear(send_sem)
                bar_in = dram_pool.tile([128, 1], f32, tag="barin")
                bar_out = dram_pool.tile([128, 1], f32, tag="barout")
                zr = state_pool.tile([128, 1], f32)
                nc.gpsimd.memset(zr[:], 0.0)
                nc.sync.dma_start(out=bar_in[:], in_=zr[:])
                barrier = nc.gpsimd.collective_compute(
                    "AllReduce", ALU.add,
                    replica_groups=[list(range(NC))],
                    ins=[bar_in[:].opt()], outs=[bar_out[:].opt()],
                )
                add_dep_helper(barrier.ins, c1.ins, sync=False,
                               reason="clear sems before barrier")
                add_dep_helper(barrier.ins, c2.ins, sync=False,
                               reason="clear sems before barrier")
                prev_ring_inst = barrier
                # the scheduler's single-core sim can't see remote increments
                # of arrive_sem, so waits are emitted with threshold 0 and
                # patched to the real value after scheduling
                wait_patches = []

            hT_prev = None
            out_sb = None
            prev_transpose = None
            for t in range(Tsteps):
                x_sb = x_pool.tile([128, 2, KC_X, B], bf, tag="x")
                nc.sync.dma_start(out=x_sb[:, 0], in_=xT[:, t])
                nc.sync.dma_start(out=x_sb[:, 1], in_=xT[:, Tsteps - 1 - t])

                z_ps = psum_pool.tile([128, SH], f32, tag="z")

                # (lhsT_f, rhs_f, lhsT_b, rhs_b) per contraction chunk
                mms = []
                for c in range(KC_X):
                    mms.append((x_sb[:, 0, c, :], wx_sb[:, 0, c, :],
                                x_sb[:, 1, c, :], wx_sb[:, 1, c, :]))
                if with_bias:
                    mms.append((ones_sb[:, 0:B], bias_sb[:, 0],
                                ones_sb[:, B:128], bias_sb[:, 1]))
                n_x_mms = len(mms)
                if t > 0:
                    for j in range(KC_H):
                        mms.append((hT_prev[:, j, 0:B], wh_sb[:, 0, j, :],
                                    hT_prev[:, j, B:128], wh_sb[:, 1, j, :]))
                # Each partition-half is its own accumulation stream into the
                # shared bank (has_written is per-element); the group checker
                # can't express that, hence skip_group_check.
                wait_inst = None
                last_x_inst = None
                for k, (lf, rf, lb, rb) in enumerate(mms):
                    if (exchange == "remote" and t > 0 and k == n_x_mms
                            and wait_inst is None):
                        # gate the h-side matmuls on all 7 remote chunk
                        # arrivals of the previous step's exchange
                        wait_inst = nc.tensor.wait_ge(arrive_sem, 0)
                        wait_patches.append((wait_inst.ins, ARR_INC * t))
                        if last_x_inst is not None:
                            add_dep_helper(wait_inst.ins, last_x_inst.ins,
                                           sync=False, reason="x-mms before arrive wait")
                        if prev_transpose is not None:
                            add_dep_helper(wait_inst.ins, prev_transpose.ins,
                                           sync=False, reason="own transpose before arrive wait")
                    first = k == 0
                    last = k == len(mms) - 1
                    m1 = nc.tensor.matmul(z_ps[0:B, :], lhsT=lf, rhs=rf,
                                          start=first, stop=last,
                                          tile_position=(0, 0), skip_group_check=True)
                    if wait_inst is not None and k == n_x_mms:
                        add_dep_helper(m1.ins, wait_inst.ins, sync=False,
                                       reason="arrive wait before h-mms")
                    nc.tensor.matmul(z_ps[B:128, :], lhsT=lb, rhs=rb,
                                     start=first, stop=last,
                                     tile_position=(0, B), skip_group_check=True)
                    if k == n_x_mms - 1:
                        last_x_inst = m1

                # gates: z cols [i | f | o | g], 128 channels each
                sig = work_pool.tile([128, 3 * CH], f32, tag="sig")
                nc.scalar.activation(sig[:], z_ps[:, 0:3 * CH], AF.Sigmoid)
                tg = work_pool.tile([128, CH], f32, tag="tg")
                nc.scalar.activation(tg[:], z_ps[:, 3 * CH:4 * CH], AF.Tanh)
                ig = work_pool.tile([128, CH], f32, tag="ig")
                nc.vector.tensor_tensor(ig[:], sig[:, 0:CH], tg[:], ALU.mult)
                nc.vector.tensor_tensor(c_st[:], sig[:, CH:2 * CH], c_st[:], ALU.mult)
                nc.vector.tensor_tensor(c_st[:], c_st[:], ig[:], ALU.add)
                tch = work_pool.tile([128, CH], f32, tag="tch")
                nc.scalar.activation(tch[:], c_st[:], AF.Tanh)

                if t % STORE_EVERY == 0:
                    out_sb = out_pool.tile([128, STORE_EVERY, CH], f32, tag="osb")
                hview = out_sb[:, t % STORE_EVERY, :]
                nc.vector.tensor_tensor(hview, sig[:, 2 * CH:3 * CH], tch[:], ALU.mult)

                if t % STORE_EVERY == STORE_EVERY - 1 or t == Tsteps - 1:
                    t0 = (t // STORE_EVERY) * STORE_EVERY
                    n = t + 1 - t0
                    nc.sync.dma_start(out=out[:, t0:t + 1, :], in_=out_sb[:, 0:n, :])

                if t < Tsteps - 1:
                    tp_ps = tpsum_pool.tile([128, 128], f32, tag="tp")
                    tp_inst = nc.tensor.transpose(tp_ps[:], hview, ident[:])
                    prev_transpose = tp_inst
                    hT_bf = work_pool.tile([128, 128], bf, tag="hTbf")
                    nc.vector.tensor_copy(hT_bf[:], tp_ps[:])
                    hT_new = hT_pool.tile([128, KC_H, 128], bf, tag="hT")
                    if exchange == "none":
                        # timing-only variant: no cross-core exchange (slots
                        # 1..7 stay stale/uninitialized — results are wrong)
                        nc.vector.tensor_copy(hT_new[:, 0, :], hT_bf[:])
                    elif exchange == "remote":
                        # own chunk lands at slot 0 locally; slot j on peer
                        # (self XOR j) receives our chunk via remote SBUF DMA
                        nc.vector.tensor_copy(hT_new[:, 0, :], hT_bf[:])
                        for j in range(1, NC):
                            rdests = [None] * NC
                            rdests[j] = (0, j)
                            prep = nc.gpsimd.remote_dma_broadcast(
                                out_ap=hT_new[:, j, :],
                                in_ap=hT_bf[:],
                                remote_sem=arrive_sem,
                                local_sem=send_sem,
                                rdests=rdests,
                            )
                            if prev_ring_inst is not None:
                                add_dep_helper(prep.ins, prev_ring_inst.ins,
                                               sync=False, reason="swdge ring order")
                            prev_ring_inst = prep
                        trig = nc.gpsimd.trigger_dma(count=None)
                        add_dep_helper(trig.ins, prev_ring_inst.ins,
                                       sync=False, reason="swdge ring order")
                        prev_ring_inst = trig
                    else:
                        cc_in = dram_pool.tile([128, 128], bf, tag="ccin")
                        cc_out = dram_pool.tile([NC * 128, 128], bf, tag="ccout")
                        nc.sync.dma_start(out=cc_in[:], in_=hT_bf[:])
                        nc.gpsimd.collective_compute(
                            "AllGather", ALU.bypass,
                            replica_groups=[list(range(NC))],
                            ins=[cc_in[:].opt()], outs=[cc_out[:].opt()],
                        )
                        if os.environ.get("LSTM_UNPACK", "multi") == "multi":
                            # 8 contiguous per-chunk unpacks (parallel DMA
                            # queues) instead of one strided rearrange DMA
                            for j in range(KC_H):
                                nc.sync.dma_start(
                                    out=hT_new[:, j, :],
                                    in_=cc_out[j * 128:(j + 1) * 128, :],
                                )
                        else:
                            nc.sync.dma_start(
                                out=hT_new[:],
                                in_=cc_out[:].rearrange("(j p) b -> p j b", p=128),
                            )
                    hT_prev = hT_new

            if exchange == "remote":
                # End barrier: every core has consumed all inbound remote
                # DMAs (its last h-mm waited on arrive_sem) before anyone
                # exits. Without this, back-to-back executions race: the
                # next run's sem_clear on core A vs core B's final sends.
                nc.sync.dma_start(out=bar_in[:], in_=zr[:])
                end_bar = nc.gpsimd.collective_compute(
                    "AllReduce", ALU.add,
                    replica_groups=[list(range(NC))],
                    ins=[bar_in[:].opt()], outs=[bar_out[:].opt()],
                )
                add_dep_helper(end_bar.ins, prev_ring_inst.ins, sync=False,
                               reason="all sends triggered before end barrier")

    if exchange == "remote":
        patched = 0
        for ins, val in wait_patches:
            waits = ins.sync_info.on_wait
            assert len(waits) >= 1 and waits[0].ant_name == "hT_arrive", (
                f"arrive wait lost its sem: {ins}"
            )
            waits[0].wait_value = val
            patched += 1
        assert patched == Tsteps - 1, (patched, Tsteps)
        # verify the patch landed in the module
        n_live = sum(
            1
            for blk in nc.m.functions[0].blocks
            for i in blk.instructions
            if i.sync_info is not None
            and any(
                w.ant_name == "hT_arrive" and w.wait_value > 0
                for w in i.sync_info.on_wait
            )
        )
        assert n_live == Tsteps - 1, (
            f"patched arrive waits not live in module: {n_live} != {Tsteps - 1}"
        )

    nc.finalize()
    return nc


def shard_inputs(inputs, Wx_f, Wh_f, b_fw, Wx_b, Wh_b, b_bw, Tsteps=T,
                 exchange="collective"):
    """Build the 8 per-core input dicts (numpy, host-side)."""
    x = np.ascontiguousarray(inputs[:, :Tsteps]).astype(BF16)        # [B,Tsteps,D]
    # xT[p, t, c, b] = x[b, t, 128c+p]
    xT = np.ascontiguousarray(
        x.transpose(2, 1, 0).reshape(KC_X, 128, Tsteps, B).transpose(1, 2, 0, 3)
    )

    with_bias = bool(np.any(b_fw) or np.any(b_bw))
    in_maps = []
    for r in range(NC):
        cols = np.concatenate(
            [np.arange(g * H + r * CH, g * H + (r + 1) * CH) for g in range(4)]
        )
        def prep_w(Wf, Wb, kc, perm=None):
            # [p, dir, chunk, col] = W_dir[128*perm(chunk) + p, cols]
            wf = Wf[:, cols].astype(BF16).reshape(kc, 128, SH)
            wb = Wb[:, cols].astype(BF16).reshape(kc, 128, SH)
            if perm is not None:
                wf, wb = wf[perm], wb[perm]
            w = np.stack([wf, wb], axis=0)          # [dir, chunk, p, col]
            return np.ascontiguousarray(w.transpose(2, 0, 1, 3))
        # remote exchange: the Q7 XORs relative dests with its PHYSICAL tpb
        # id; on trn2 logical<->physical is pi(x) = x^2 for x>=4 (measured
        # with probe_map.py), so receiver r's slot j holds sender
        # pi(pi(r)^j)'s channels. Permute Wh's row-chunks to slot order.
        pi = lambda x: x ^ 2 if x & 4 else x
        hperm = (
            np.array([pi(pi(r) ^ j) for j in range(KC_H)])
            if exchange == "remote" else None
        )
        m = {
            "xT": xT,
            "wh": prep_w(Wh_f, Wh_b, KC_H, hperm),
            "wx": prep_w(Wx_f, Wx_b, KC_X),
        }
        if with_bias:
            m["bias"] = np.ascontiguousarray(
                np.stack([b_fw[cols], b_bw[cols]], axis=0)[None].astype(BF16)
            )
        in_maps.append(m)
    return in_maps, with_bias


_NC_CACHE = {}


# "collective" (ncfw AllGather per step, ~10.8 ms device time, stable across
# repeated executions) or "remote" (SBUF->SBUF remote DMA exchange — ~2x
# faster per step but the axon/SWDGE stack desyncs the whole mesh
# unrecoverably on ~1 in 5-15 executions, so it is opt-in only)
EXCHANGE = os.environ.get("LSTM_EXCHANGE", "collective")


def run(inputs, Wx_f, Wh_f, b_fw, Wx_b, Wh_b, b_bw, Tsteps=T, trace=False,
        exchange=None):
    exchange = EXCHANGE if exchange is None else exchange
    in_maps, with_bias = shard_inputs(
        inputs, Wx_f, Wh_f, b_fw, Wx_b, Wh_b, b_bw, Tsteps, exchange
    )
    key = (Tsteps, with_bias, exchange)
    if key not in _NC_CACHE:
        _NC_CACHE[key] = build_nc(Tsteps, with_bias, exchange)
    nc = _NC_CACHE[key]
    res = run_bass_kernel_spmd(
        nc, in_maps, core_ids=list(range(NC)), trace=trace,
    )
    full = np.empty((B, Tsteps, 2 * H), np.float32)
    for r in range(NC):
        o = res.results[r]["out"]                  # [128, Tsteps, CH] f32
        full[:, :, r * CH:(r + 1) * CH] = o[0:B]
        full[:, :, H + r * CH:H + (r + 1) * CH] = o[B:128][:, ::-1, :]
    return full, res


def kernel(**inputs) -> np.ndarray:
    args = (
        np.asarray(inputs["inputs"], np.float32),
        np.asarray(inputs["Wx_f"], np.float32),
        np.asarray(inputs["Wh_f"], np.float32),
        np.asarray(inputs["b_fw"], np.float32),
        np.asarray(inputs["Wx_b"], np.float32),
        np.asarray(inputs["Wh_b"], np.float32),
        np.asarray(inputs["b_bw"], np.float32),
    )
    try:
        out, _ = run(*args)
        return out
    except Exception:
        # the remote-DMA exchange has shown rare first-exec "mesh desynced"
        # failures; retry once on the collective path
        if EXCHANGE == "collective":
            raise
        out, _ = run(*args, exchange="collective")
        return out



# revision 4
# speedup vs baseline: 3.7370x; 3.7370x over previous
"""Bidirectional LSTM v3: single packed AllGather per step, lean DMA chain.

vs baseline: bf16 output (halved store traffic), 4-step-batched x loads on the
ACT queue, ONE strided unpack DMA instead of 8, wide [128,*] gate ops, same
packed fwd/bwd PSUM layout. vs v2: collectives serialize on the Pool queue,
so one [128,128]-oriented AG per step (fast orientation) beats two.
"""

import numpy as np
import ml_dtypes

import concourse.bacc as bacc
import concourse.mybir as mybir
from concourse.tile import TileContext
from concourse.tile_rust import add_dep_helper
from concourse.masks import make_identity
from concourse.bass_utils import run_bass_kernel_spmd

BF16 = ml_dtypes.bfloat16

B, T, D, H = 64, 512, 512, 1024
NC = 8
CH = H // NC
SH = 4 * CH
KC_H = H // 128
KC_X = D // 128
SE = 8                  # h steps buffered per out-DMA
XB = 4                  # x prefetch block

bf = mybir.dt.bfloat16
f32 = mybir.dt.float32
AF = mybir.ActivationFunctionType
ALU = mybir.AluOpType


def build_nc(Tsteps=T, with_bias=False, exchange="collective", fill=8):
    nc = bacc.Bacc(None, target_bir_lowering=False, debug=False,
                   num_devices=NC)

    xT = nc.declare_dram_parameter("xT", [128, Tsteps, KC_X, B], bf,
                                   isOutput=False)
    wh = nc.declare_dram_parameter("wh", [128, 2, KC_H, SH], bf,
                                   isOutput=False)
    wx = nc.declare_dram_parameter("wx", [128, 2, KC_X, SH], bf,
                                   isOutput=False)
    if with_bias:
        bias = nc.declare_dram_parameter("bias", [1, 2, SH], bf,
                                         isOutput=False)
    out = nc.declare_dram_parameter("out", [128, Tsteps, CH], bf,
                                    isOutput=True)

    with TileContext(nc) as tc:
        with (
            tc.tile_pool(name="const", bufs=1) as const_pool,
            tc.tile_pool(name="state", bufs=1) as state_pool,
            tc.tile_pool(name="xin", bufs=2) as x_pool,
            tc.tile_pool(name="work", bufs=2) as work_pool,
            tc.tile_pool(name="hT", bufs=2) as hT_pool,
            tc.tile_pool(name="zps", bufs=2, space="PSUM") as psum_pool,
            tc.tile_pool(name="tps", bufs=2, space="PSUM") as tpsum_pool,
            tc.tile_pool(name="outb", bufs=2) as out_pool,
            tc.tile_pool(name="dram", bufs=2, space="DRAM") as dram_pool,
        ):
            wh_sb = const_pool.tile([128, 2, KC_H, SH], bf)
            nc.sync.dma_start(out=wh_sb[:], in_=wh[:, :, :, :])
            wx_sb = const_pool.tile([128, 2, KC_X, SH], bf)
            nc.sync.dma_start(out=wx_sb[:], in_=wx[:, :, :, :])
            ident = const_pool.tile([128, 128], f32)
            make_identity(nc, ident[:])
            ident_bf = const_pool.tile([128, 128], bf)
            nc.vector.tensor_copy(ident_bf[:], ident[:])
            if with_bias:
                ones_sb = const_pool.tile([1, 128], bf)
                nc.gpsimd.memset(ones_sb[:], 1.0)
                bias_sb = const_pool.tile([1, 2, SH], bf)
                nc.sync.dma_start(out=bias_sb[:], in_=bias[:, :, :])

            c_st = state_pool.tile([128, CH], f32)
            nc.gpsimd.memset(c_st[:], 0.0)

            # Dense matmul prologue: trips the PE HAM busy window so the
            # kernel starts (and stays, via per-step fill) at 2.4 GHz
            # regardless of the HAM phase at NEFF entry. ~30 us once.
            if fill:
                wu = tpsum_pool.tile([128, SH], f32, tag="fill")
                for i in range(128):
                    nc.tensor.matmul(
                        wu[:], lhsT=wh_sb[:, 0, i % KC_H, 0:128],
                        rhs=wh_sb[:, 1, i % KC_H, :],
                        start=True, stop=True, tile_position=(0, 0),
                    )

            hT_prev = None
            out_sb = None
            xblk = None
            for t in range(Tsteps):
                if t % XB == 0:
                    xblk = x_pool.tile([128, 2, XB, KC_X, B], bf, tag="x")
                    nc.scalar.dma_start(out=xblk[:, 0], in_=xT[:, t:t + XB])
                    nc.scalar.dma_start(
                        out=xblk[:, 1],
                        in_=xT[:, Tsteps - t - XB:Tsteps - t])
                if t % SE == 0:
                    out_sb = out_pool.tile([128, SE, CH], bf, tag="osb")

                xi_f = t % XB
                xi_b = XB - 1 - (t % XB)
                z_ps = psum_pool.tile([128, SH], f32, tag="z")

                mms = []
                for c in range(KC_X):
                    mms.append((xblk[:, 0, xi_f, c, :], wx_sb[:, 0, c, :],
                                xblk[:, 1, xi_b, c, :], wx_sb[:, 1, c, :]))
                if with_bias:
                    mms.append((ones_sb[:, 0:B], bias_sb[:, 0],
                                ones_sb[:, B:128], bias_sb[:, 1]))
                if t > 0:
                    for j in range(KC_H):
                        mms.append((hT_prev[:, j, 0:B], wh_sb[:, 0, j, :],
                                    hT_prev[:, j, B:128], wh_sb[:, 1, j, :]))
                for k, (lf, rf, lb, rb) in enumerate(mms):
                    first = k == 0
                    last = k == len(mms) - 1
                    nc.tensor.matmul(z_ps[0:B, :], lhsT=lf, rhs=rf,
                                     start=first, stop=last,
                                     tile_position=(0, 0),
                                     skip_group_check=True)
                    nc.tensor.matmul(z_ps[B:128, :], lhsT=lb, rhs=rb,
                                     start=first, stop=last,
                                     tile_position=(0, B),
                                     skip_group_check=True)

                # gates: z cols [i | f | o | g]
                sig = work_pool.tile([128, 3 * CH], f32, tag="sig")
                nc.scalar.activation(sig[:], z_ps[:, 0:3 * CH], AF.Sigmoid)
                tg = work_pool.tile([128, CH], f32, tag="tg")
                nc.scalar.activation(tg[:], z_ps[:, 3 * CH:4 * CH], AF.Tanh)
                ig = work_pool.tile([128, CH], f32, tag="ig")
                nc.vector.tensor_tensor(ig[:], sig[:, 0:CH], tg[:], ALU.mult)
                nc.vector.tensor_tensor(c_st[:], sig[:, CH:2 * CH], c_st[:],
                                        ALU.mult)
                nc.vector.tensor_tensor(c_st[:], c_st[:], ig[:], ALU.add)
                tch = work_pool.tile([128, CH], f32, tag="tch")
                nc.scalar.activation(tch[:], c_st[:], AF.Tanh)

                hview = out_sb[:, t % SE, :]                 # bf16 [128,128]
                nc.vector.tensor_tensor(hview, sig[:, 2 * CH:3 * CH], tch[:],
                                        ALU.mult)

                if t % SE == SE - 1 or t == Tsteps - 1:
                    t0 = (t // SE) * SE
                    nc.sync.dma_start(out=out[:, t0:t + 1, :],
                                      in_=out_sb[:, 0:t + 1 - t0, :])

                if t < Tsteps - 1:
                    tp_ps = tpsum_pool.tile([128, 128], bf, tag="tp")
                    nc.tensor.transpose(tp_ps[:], hview, ident_bf[:])
                    hT_bf = work_pool.tile([128, 128], bf, tag="hTbf")
                    nc.vector.tensor_copy(hT_bf[:], tp_ps[:])
                    cc_in = dram_pool.tile([128, 128], bf, tag="ccin")
                    ci = nc.scalar.dma_start(out=cc_in[:], in_=hT_bf[:])
                    cc_out = dram_pool.tile([NC * 128, 128], bf, tag="ccout")
                    if exchange == "collective":
                        nc.gpsimd.collective_compute(
                            "AllGather", ALU.bypass,
                            replica_groups=[list(range(NC))],
                            ins=[cc_in[:].opt()], outs=[cc_out[:].opt()],
                        )
                    hT_new = hT_pool.tile([128, KC_H, 128], bf, tag="hT")
                    # 8 contiguous per-chunk unpacks, split across the SP and
                    # ACT DGE queues so the ~0.6us setups overlap
                    for j in range(KC_H):
                        eng = nc.sync if j % 2 == 0 else nc.scalar
                        up = eng.dma_start(
                            out=hT_new[:, j, :],
                            in_=cc_out[j * 128:(j + 1) * 128, :],
                        )
                        if exchange == "none":
                            add_dep_helper(up.ins, ci.ins, sync=True,
                                           reason="skeleton chain")
                    hT_prev = hT_new

                    if fill:
                        # PE-warming filler: keeps the HAM clock gate at 2.4
                        # GHz through the exchange wait. Reads resident
                        # weights, writes a scratch PSUM tile nobody reads.
                        scr = tpsum_pool.tile([128, SH], f32, tag="fill")
                        for i in range(fill):
                            nc.tensor.matmul(
                                scr[:], lhsT=wh_sb[:, 0, i % KC_H, 0:128],
                                rhs=wh_sb[:, 1, i % KC_H, :],
                                start=True, stop=True, tile_position=(0, 0),
                            )

    nc.finalize()
    return nc


def shard_inputs(inputs, Wx_f, Wh_f, b_fw, Wx_b, Wh_b, b_bw, Tsteps=T):
    x = np.ascontiguousarray(inputs[:, :Tsteps]).astype(BF16)
    xT = np.ascontiguousarray(
        x.transpose(2, 1, 0).reshape(KC_X, 128, Tsteps, B).transpose(1, 2, 0, 3)
    )
    with_bias = bool(np.any(b_fw) or np.any(b_bw))
    in_maps = []
    for r in range(NC):
        cols = np.concatenate(
            [np.arange(g * H + r * CH, g * H + (r + 1) * CH) for g in range(4)]
        )

        def prep_w(Wf, Wb, kc):
            wf = Wf[:, cols].astype(BF16).reshape(kc, 128, SH)
            wb = Wb[:, cols].astype(BF16).reshape(kc, 128, SH)
            w = np.stack([wf, wb], axis=0)
            return np.ascontiguousarray(w.transpose(2, 0, 1, 3))

        m = {
            "xT": xT,
            "wh": prep_w(Wh_f, Wh_b, KC_H),
            "wx": prep_w(Wx_f, Wx_b, KC_X),
        }
        if with_bias:
            m["bias"] = np.ascontiguousarray(
                np.stack([b_fw[cols], b_bw[cols]], axis=0)[None].astype(BF16)
            )
        in_maps.append(m)
    return in_maps, with_bias


_NC_CACHE = {}


def run(inputs, Wx_f, Wh_f, b_fw, Wx_b, Wh_b, b_bw, Tsteps=T, trace=False):
    in_maps, with_bias = shard_inputs(
        inputs, Wx_f, Wh_f, b_fw, Wx_b, Wh_b, b_bw, Tsteps
    )
    key = (Tsteps, with_bias)
    if key not in _NC_CACHE:
        _NC_CACHE[key] = build_nc(Tsteps, with_bias)
    nc = _NC_CACHE[key]
    res = run_bass_kernel_spmd(
        nc, in_maps, core_ids=list(range(NC)), trace=trace,
    )
    full = np.empty((B, Tsteps, 2 * H), np.float32)
    for r in range(NC):
        o = np.asarray(res.results[r]["out"]).astype(np.float32)
        full[:, :, r * CH:(r + 1) * CH] = o[0:B]
        full[:, :, H + r * CH:H + (r + 1) * CH] = o[B:128][:, ::-1, :]
    return full, res


def kernel(**inputs) -> np.ndarray:
    args = (
        np.asarray(inputs["inputs"], np.float32),
        np.asarray(inputs["Wx_f"], np.float32),
        np.asarray(inputs["Wh_f"], np.float32),
        np.asarray(inputs["b_fw"], np.float32),
        np.asarray(inputs["Wx_b"], np.float32),
        np.asarray(inputs["Wh_b"], np.float32),
        np.asarray(inputs["b_bw"], np.float32),
    )
    out, _ = run(*args)
    return out
